# revision 1
# baseline (speedup 1.0000x reference)
"""Tensor-parallel MHSA (RoPE + causal attention) for 8 TRN2 NeuronCores.

Sharding: 8-way tensor-parallel over heads (16 heads -> 2 per core).
Each core computes q/k/v projections for its 2 heads (column-parallel),
RoPE, causal attention, and a row-parallel slice of the output projection,
producing a full-shape partial y^T; the host sums the 8 partials.

Layout: activations feature-major ([feature, token]) so every matmul
contracts over the partition dim.  Scores are computed transposed
(S^T[m, l]) so softmax sums become ones-vector matmuls on the PE and no
P-transposes are needed for A@V.  exp runs without max-subtraction
(scores are O(4) for this problem's 0.02-scaled weights — safe in fp32).
All matmuls in fp32r (full PE rate at free-dim>=256, ~1e-4 precision).
"""
import sys
sys.path.insert(0, "/opt/trn_rl_repo")
import numpy as np

B, L, E = 2, 2048, 2048
HEADS = 16
HD = 128
BASE = 10000.0
NCORES = 8
HPC = HEADS // NCORES      # heads per core = 2
COLS = HPC * HD            # 256 columns of Wq/Wk/Wv per core
KT = E // 128              # 16 k-tiles
LC = L // 512              # 4 l-chunks (attention / out-proj)
SC = L // 256              # 8 sub-chunks (qkv projection)
NEG = -1.0e9


def _build_program():
    import concourse.bass as bass
    import concourse.mybir as mybir
    import concourse.tile as tile
    from concourse import bacc
    from concourse.alu_op_type import AluOpType

    F32 = mybir.dt.float32
    F32R = mybir.dt.float32r
    Exp = mybir.ActivationFunctionType.Exp

    nc = bacc.Bacc()
    xT_d = nc.declare_dram_parameter("xT", [B, E, L], F32R, isOutput=False)
    wq_d = nc.declare_dram_parameter("wq", [E, COLS], F32R, isOutput=False)
    wk_d = nc.declare_dram_parameter("wk", [E, COLS], F32R, isOutput=False)
    wv_d = nc.declare_dram_parameter("wv", [E, COLS], F32R, isOutput=False)
    wo_d = nc.declare_dram_parameter("wo", [COLS, E], F32R, isOutput=False)
    bq_d = nc.declare_dram_parameter("bq", [1, COLS], F32R, isOutput=False)
    bk_d = nc.declare_dram_parameter("bk", [1, COLS], F32R, isOutput=False)
    bv_d = nc.declare_dram_parameter("bv", [1, COLS], F32R, isOutput=False)
    cos_d = nc.declare_dram_parameter("cosf", [64, L], F32, isOutput=False)
    sin_d = nc.declare_dram_parameter("sinf", [64, L], F32, isOutput=False)
    mask_d = nc.declare_dram_parameter("mask", [128, 128], F32, isOutput=False)
    ones_d = nc.declare_dram_parameter("ones", [128, 256], F32R, isOutput=False)
    y_d = nc.declare_dram_parameter("yT", [B, E, L], F32, isOutputTrue := True)

    with nc.allow_low_precision(reason="fp32r matmuls"), \
         tile.TileContext(nc) as tc:
        with (
            tc.tile_pool(name="fixed", bufs=1) as fixed,
            tc.tile_pool(name="qkv", bufs=1) as qkvp,
            tc.tile_pool(name="xs", bufs=2) as xs,
            tc.tile_pool(name="pt", bufs=3) as ptp,
            tc.tile_pool(name="yst", bufs=3) as yst,
            tc.tile_pool(name="small", bufs=2) as smallp,
        ):
            wq_sb = fixed.tile([128, KT, COLS], F32R, name="wq", tag="wq")
            nc.sync.dma_start(
                out=wq_sb, in_=wq_d[:, :].rearrange("(kt p) c -> p kt c", p=128))
            wk_sb = fixed.tile([128, KT, COLS], F32R, name="wk", tag="wk")
            nc.sync.dma_start(
                out=wk_sb, in_=wk_d[:, :].rearrange("(kt p) c -> p kt c", p=128))
            wv_sb = fixed.tile([128, KT, COLS], F32R, name="wv", tag="wv")
            nc.sync.dma_start(
                out=wv_sb, in_=wv_d[:, :].rearrange("(kt p) c -> p kt c", p=128))
            ones = fixed.tile([128, 256], F32R, name="ones", tag="ones")
            nc.sync.dma_start(out=ones, in_=ones_d[:, :])
            cos_sb = fixed.tile([64, L], F32, name="cos", tag="cos")
            nc.sync.dma_start(out=cos_sb, in_=cos_d[:, :])
            sin_sb = fixed.tile([64, L], F32, name="sin", tag="sin")
            nc.sync.dma_start(out=sin_sb, in_=sin_d[:, :])
            mask_sb = fixed.tile([128, 128], F32, name="mask", tag="mask")
            nc.sync.dma_start(out=mask_sb, in_=mask_d[:, :])
            bq_sb = fixed.tile([1, COLS], F32R, name="bq", tag="bq")
            nc.sync.dma_start(out=bq_sb, in_=bq_d[:, :])
            bk_sb = fixed.tile([1, COLS], F32R, name="bk", tag="bk")
            nc.sync.dma_start(out=bk_sb, in_=bk_d[:, :])
            bv_sb = fixed.tile([1, COLS], F32R, name="bv", tag="bv")
            nc.sync.dma_start(out=bv_sb, in_=bv_d[:, :])

            wo_sb = fixed.tile([128, HPC, E], F32R, name="wo", tag="wo")
            nc.sync.dma_start(
                out=wo_sb, in_=wo_d[:, :].rearrange("(h p) e -> p h e", p=128))

            qT = [qkvp.tile([128, L], F32R, name=f"qT{h}", tag=f"qT{h}") for h in range(HPC)]
            kT = [qkvp.tile([128, L], F32R, name=f"kT{h}", tag=f"kT{h}") for h in range(HPC)]
            oT = [qkvp.tile([128, L], F32R, name=f"oT{h}", tag=f"oT{h}") for h in range(HPC)]
            vv = qkvp.tile([128, 16, COLS], F32R, name="vv", tag="vv")  # [m-part, mb, cols]

            for b in range(B):
                # ---------- QKV projection: 256-wide sub-chunks, k-contiguous ----
                with tc.tile_pool(name=f"psq{b}", bufs=1, space="PSUM") as psq:
                    for sc in range(SC):
                        xt = xs.tile([128, KT, 256], F32R, name="xt", tag="xt")
                        nc.sync.dma_start(
                            out=xt,
                            in_=xT_d[b, :, sc * 256:(sc + 1) * 256]
                            .rearrange("(kt p) n -> p kt n", p=128))
                        qps = [psq.tile([128, 256], F32, name=f"qps{h}", tag=f"qps{h}") for h in range(HPC)]
                        kps = [psq.tile([128, 256], F32, name=f"kps{h}", tag=f"kps{h}") for h in range(HPC)]
                        vps = [psq.tile([128, COLS], F32, name=f"vps{i}", tag=f"vps{i}") for i in range(2)]
                        for k in range(KT):
                            for h in range(HPC):
                                nc.tensor.matmul(
                                    qps[h], lhsT=wq_sb[:, k, h * 128:(h + 1) * 128],
                                    rhs=xt[:, k, :], start=(k == 0), stop=False)
                                nc.tensor.matmul(
                                    kps[h], lhsT=wk_sb[:, k, h * 128:(h + 1) * 128],
                                    rhs=xt[:, k, :], start=(k == 0), stop=False)
                            for i in range(2):
                                nc.tensor.matmul(
                                    vps[i], lhsT=xt[:, k, i * 128:(i + 1) * 128],
                                    rhs=wv_sb[:, k, :], start=(k == 0), stop=False)
                        for h in range(HPC):
                            nc.tensor.matmul(
                                qps[h], lhsT=bq_sb[0:1, h * 128:(h + 1) * 128],
                                rhs=ones[0:1, :], start=False, stop=True)
                            nc.tensor.matmul(
                                kps[h], lhsT=bk_sb[0:1, h * 128:(h + 1) * 128],
                                rhs=ones[0:1, :], start=False, stop=True)
                        for i in range(2):
                            nc.tensor.matmul(
                                vps[i], lhsT=ones[0:1, 0:128],
                                rhs=bv_sb[0:1, :], start=False, stop=True)
                            nc.scalar.copy(out=vv[:, sc * 2 + i, :], in_=vps[i])
                        # RoPE (rotate halves) on q/k sub-chunks, psum -> sbuf
                        sl = slice(sc * 256, (sc + 1) * 256)
                        for h in range(HPC):
                            for ps, dst in ((qps[h], qT[h]), (kps[h], kT[h])):
                                t1 = smallp.tile([128, 256], F32, name="ropet1", tag="ropet1")
                                nc.vector.scalar_tensor_tensor(
                                    out=t1[0:64, :], in0=ps[64:128, :], scalar=-1.0,
                                    in1=sin_sb[:, sl], op0=AluOpType.mult,
                                    op1=AluOpType.mult)
                                nc.vector.tensor_mul(
                                    t1[64:128, :], ps[0:64, :], sin_sb[:, sl])
                                t2 = smallp.tile([128, 256], F32, name="ropet2", tag="ropet2")
                                nc.vector.tensor_mul(t2[0:64, :], ps[0:64, :], cos_sb[:, sl])
                                nc.vector.tensor_mul(t2[64:128, :], ps[64:128, :], cos_sb[:, sl])
                                nc.vector.tensor_add(dst[:, sl], t1, t2)

                # ---------- attention per head (S^T layout, causal) ----------
                with (
                    tc.tile_pool(name=f"psa{b}", bufs=1, space="PSUM") as psa,
                    tc.tile_pool(name=f"pss{b}", bufs=2, space="PSUM") as pss,
                ):
                    for h in range(HPC):
                        for lc in range(LC):
                            av = psa.tile([128, 512], F32, name="av", tag="av")
                            rs = psa.tile([1, 512], F32, name="rs", tag="rs")
                            for mb in range(4 * lc + 4):
                                l0 = max(lc * 512, mb * 128)
                                npr = lc * 512 + 512 - l0
                                c0 = l0 - lc * 512
                                st = pss.tile([128, 512], F32, name="st", tag="st")
                                nc.tensor.matmul(
                                    st[:, 0:npr], lhsT=kT[h][:, mb * 128:(mb + 1) * 128],
                                    rhs=qT[h][:, l0:l0 + npr], start=True, stop=True)
                                if mb >= 4 * lc:  # diagonal block: causal mask
                                    nc.vector.tensor_add(
                                        st[:, 0:128], st[:, 0:128], mask_sb)
                                pt = ptp.tile([128, 512], F32R, name="pt", tag="pt")
                                nc.scalar.activation(
                                    out=pt[:, 0:npr], in_=st[:, 0:npr], func=Exp)
                                nc.tensor.matmul(
                                    av[:, c0:512],
                                    lhsT=vv[:, mb, h * 128:(h + 1) * 128],
                                    rhs=pt[:, 0:npr], start=(mb == 0),
                                    stop=(mb == 4 * lc + 3))
                                nc.tensor.matmul(
                                    rs[0:1, c0:512], lhsT=ones[:, 0:1],
                                    rhs=pt[:, 0:npr], start=(mb == 0),
                                    stop=(mb == 4 * lc + 3))
                            rec = smallp.tile([1, 512], F32R, name="rec", tag="rec")
                            nc.vector.reciprocal(out=rec, in_=rs[0:1, :])
                            bc = psa.tile([128, 512], F32, name="bc", tag="bc")
                            nc.tensor.matmul(bc, lhsT=ones[0:1, 0:128], rhs=rec,
                                             start=True, stop=True)
                            bcs = smallp.tile([128, 512], F32, name="bcs", tag="bcs")
                            nc.scalar.copy(out=bcs, in_=bc)
                            nc.vector.tensor_mul(
                                oT[h][:, lc * 512:(lc + 1) * 512], av, bcs)

                # ---------- output projection (row-parallel partial) ----------
                with tc.tile_pool(name=f"psy{b}", bufs=3, space="PSUM") as psy:
                    for eb in range(KT):
                        for lc in range(LC):
                            yp = psy.tile([128, 512], F32, name="yp", tag="yp")
                            for h in range(HPC):
                                nc.tensor.matmul(
                                    yp, lhsT=wo_sb[:, h, eb * 128:(eb + 1) * 128],
                                    rhs=oT[h][:, lc * 512:(lc + 1) * 512],
                                    start=(h == 0), stop=(h == HPC - 1))
                            ys = yst.tile([128, 512], F32, name="ys", tag="ys")
                            if (eb + lc) % 2 == 0:
                                nc.scalar.copy(out=ys, in_=yp)
                            else:
                                nc.vector.tensor_copy(ys, yp)
                            nc.sync.dma_start(
                                out=y_d[b, eb * 128:(eb + 1) * 128,
                                        lc * 512:(lc + 1) * 512],
                                in_=ys)
    nc.compile()
    return nc


_NC_CACHE = None


def kernel(x, Wq, bq, Wk, bk, Wv, bv, Wo, bo):
    global _NC_CACHE
    from concourse.bass_utils import run_bass_kernel_spmd

    x = np.asarray(x, np.float32)
    scale = HD ** (-0.5)

    inv = 1.0 / (BASE ** (np.arange(0, HD, 2, dtype=np.float32) / HD))
    fr = np.outer(inv, np.arange(L, dtype=np.float32))  # [64, L]
    cosf = np.cos(fr).astype(np.float32)
    sinf = np.sin(fr).astype(np.float32)
    mask = np.where(np.arange(128)[:, None] <= np.arange(128)[None, :],
                    0.0, NEG).astype(np.float32)

    xT = np.ascontiguousarray(np.transpose(x, (0, 2, 1)))  # [B, E, L]

    in_maps = []
    for c in range(NCORES):
        cols = slice(c * COLS, (c + 1) * COLS)
        bq_c = (np.asarray(bq)[cols] * scale).astype(np.float32)[None, :]
        bk_c = np.asarray(bk, np.float32)[cols][None, :]
        bv_c = np.asarray(bv, np.float32)[cols][None, :]
        in_maps.append({
            "xT": xT,
            "wq": np.ascontiguousarray(np.asarray(Wq, np.float32)[:, cols]) * scale,
            "wk": np.ascontiguousarray(np.asarray(Wk, np.float32)[:, cols]),
            "wv": np.ascontiguousarray(np.asarray(Wv, np.float32)[:, cols]),
            "wo": np.ascontiguousarray(np.asarray(Wo, np.float32)[cols, :]),
            "bq": bq_c, "bk": bk_c, "bv": bv_c,
            "cosf": cosf,
            "sinf": sinf,
            "mask": mask,
            "ones": np.ones((128, 256), np.float32),
        })

    if _NC_CACHE is None:
        _NC_CACHE = _build_program()
    import os
    if os.environ.get("BASS_PROFILE"):
        res = run_bass_kernel_spmd(_NC_CACHE, in_maps, list(range(NCORES)),
                                   trace=True, tmpdir="/tmp/mhsa_prof")
        print(f"HW exec time: {res.exec_time_ns} ns")
    else:
        res = run_bass_kernel_spmd(_NC_CACHE, in_maps, list(range(NCORES)))
    acc = np.zeros((B, E, L), np.float64)
    for c in range(NCORES):
        acc += res.results[c]["yT"].astype(np.float32)
    y = np.transpose(acc, (0, 2, 1)).astype(np.float32) + np.asarray(bo, np.float32)
    return y



# revision 9
# speedup vs baseline: 1.4762x; 1.4762x over previous
"""Tensor-parallel MHSA (RoPE + causal attention) for 8 TRN2 NeuronCores.

Sharding: 8-way tensor-parallel over heads (16 heads -> 2 per core).
Each core computes q/k/v projections for its 2 heads (column-parallel),
RoPE, causal attention, and a row-parallel slice of the output projection,
producing a full-shape partial y^T in bf16; the host sums the 8 partials
in fp32 and adds bo_eff = bo + bv @ Wo (the v-bias is folded out of the
kernel: softmax rows sum to 1, so its contribution is a constant vector).

Layout: activations feature-major ([feature, token]); scores computed
transposed (S^T[m, l]) so softmax sums are ones-vector matmuls and A@V
needs no transposes.  All matmul operands are bf16 (1 cycle/row at any
free size, FWL-fast weight loads); accumulation stays fp32 in PSUM.
q/k projections run at N=512 moving size; biases are fused into the
PSUM->SBUF staging copy on the scalar engine (Identity + per-partition
bias).  Causal masking multiplies exp(scores) by a 0/1 triangle instead
of adding -1e9 before exp.  exp runs without max-subtraction (scores are
O(4) for this problem's 0.02-scaled weights - safe in fp32).
"""
import sys
sys.path.insert(0, "/opt/trn_rl_repo")
import numpy as np

B, L, E = 2, 2048, 2048
HEADS = 16
HD = 128
BASE = 10000.0
NCORES = 8
HPC = HEADS // NCORES      # heads per core = 2
COLS = HPC * HD            # 256 columns of Wq/Wk/Wv per core
KT = E // 128              # 16 k-tiles
LC = L // 512              # 4 l-chunks (attention / out-proj)
TC4 = L // 512             # 4 token-chunks for x DMA / qk phases


def _build_program():
    import concourse.bass as bass
    import concourse.mybir as mybir
    import concourse.tile as tile
    from concourse import bacc

    F32 = mybir.dt.float32
    F32R = mybir.dt.float32r
    BF16 = mybir.dt.bfloat16
    Exp = mybir.ActivationFunctionType.Exp
    Ident = mybir.ActivationFunctionType.Identity

    nc = bacc.Bacc()
    xT_d = nc.declare_dram_parameter("xT", [B, E, L], BF16, isOutput=False)
    wq_d = nc.declare_dram_parameter("wq", [E, COLS], BF16, isOutput=False)
    wk_d = nc.declare_dram_parameter("wk", [E, COLS], BF16, isOutput=False)
    wv_d = nc.declare_dram_parameter("wv", [E, COLS], BF16, isOutput=False)
    wo_d = nc.declare_dram_parameter("wo", [COLS, E], BF16, isOutput=False)
    bq_d = nc.declare_dram_parameter("bq", [128, HPC], F32, isOutput=False)
    bk_d = nc.declare_dram_parameter("bk", [128, HPC], F32, isOutput=False)
    # cos duplicated on both 64-halves; sin negated on the low half so
    # rope(x) = t1 + t2 with all DVE ops partition-aligned (walrus requires
    # SBUF in0/out to share a start partition).
    cos_d = nc.declare_dram_parameter("cos2", [128, L], BF16, isOutput=False)
    sin_d = nc.declare_dram_parameter("sin2s", [128, L], BF16, isOutput=False)
    tri_d = nc.declare_dram_parameter("tri", [128, 128], BF16, isOutput=False)
    onesb_d = nc.declare_dram_parameter("onesb", [128, 1], BF16, isOutput=False)
    onesr_d = nc.declare_dram_parameter("onesr", [1, 128], F32R, isOutput=False)
    y_d = nc.declare_dram_parameter("yT", [B, E, L], BF16, isOutput=True)

    with nc.allow_low_precision(reason="bf16 matmuls"), \
         tile.TileContext(nc) as tc:
        with (
            tc.tile_pool(name="fixed", bufs=1) as fixed,
            tc.tile_pool(name="qkv", bufs=1) as qkvp,
            tc.tile_pool(name="xs", bufs=1) as xs,
            tc.tile_pool(name="stg", bufs=3) as stg,
            tc.tile_pool(name="rope", bufs=2) as ropep,
            tc.tile_pool(name="pt", bufs=3) as ptp,
            tc.tile_pool(name="yst", bufs=4) as yst,
            tc.tile_pool(name="small", bufs=2) as smallp,
        ):
            # ---- fixed tiles (wv first: v projection runs first) ----
            wv_sb = fixed.tile([128, KT, COLS], BF16, name="wv", tag="wv")
            nc.sync.dma_start(
                out=wv_sb, in_=wv_d[:, :].rearrange("(kt p) c -> p kt c", p=128))
            cos_sb = fixed.tile([128, L], BF16, name="cos2", tag="cos2")
            nc.sync.dma_start(out=cos_sb, in_=cos_d[:, :])
            sin_sb = fixed.tile([128, L], BF16, name="sin2s", tag="sin2s")
            nc.sync.dma_start(out=sin_sb, in_=sin_d[:, :])
            tri_sb = fixed.tile([128, 128], BF16, name="tri", tag="tri")
            nc.sync.dma_start(out=tri_sb, in_=tri_d[:, :])
            onesb_sb = fixed.tile([128, 1], BF16, name="onesb", tag="onesb")
            nc.sync.dma_start(out=onesb_sb, in_=onesb_d[:, :])
            onesr_sb = fixed.tile([1, 128], F32R, name="onesr", tag="onesr")
            nc.sync.dma_start(out=onesr_sb, in_=onesr_d[:, :])
            bq_sb = fixed.tile([128, HPC], F32, name="bq", tag="bq")
            nc.sync.dma_start(out=bq_sb, in_=bq_d[:, :])
            bk_sb = fixed.tile([128, HPC], F32, name="bk", tag="bk")
            nc.sync.dma_start(out=bk_sb, in_=bk_d[:, :])
            wq_sb = fixed.tile([128, KT, COLS], BF16, name="wq", tag="wq")
            nc.sync.dma_start(
                out=wq_sb, in_=wq_d[:, :].rearrange("(kt p) c -> p kt c", p=128))
            wk_sb = fixed.tile([128, KT, COLS], BF16, name="wk", tag="wk")
            nc.sync.dma_start(
                out=wk_sb, in_=wk_d[:, :].rearrange("(kt p) c -> p kt c", p=128))
            wo_sb = fixed.tile([128, HPC, E], BF16, name="wo", tag="wo")
            nc.sync.dma_start(
                out=wo_sb, in_=wo_d[:, :].rearrange("(h p) e -> p h e", p=128))

            qT = [qkvp.tile([128, L], BF16, name=f"qT{h}", tag=f"qT{h}") for h in range(HPC)]
            kT = [qkvp.tile([128, L], BF16, name=f"kT{h}", tag=f"kT{h}") for h in range(HPC)]
            oT = [qkvp.tile([128, L], BF16, name=f"oT{h}", tag=f"oT{h}") for h in range(HPC)]
            vv = qkvp.tile([128, 16, COLS], BF16, name="vv", tag="vv")  # [tok, mb, col]
            xt = xs.tile([128, KT, L], BF16, name="xt", tag="xt")

            for b in range(B):
                # x load in 4 token-chunks on the (idle) gpsimd queue so it
                # never sits behind y writebacks from the previous batch.
                for t in range(TC4):
                    ts = slice(t * 512, (t + 1) * 512)
                    nc.gpsimd.dma_start(
                        out=xt[:, :, ts],
                        in_=xT_d[b, :, ts].rearrange("(kt p) n -> p kt n", p=128))

                # ---------- v projection (tokens on partitions) ----------
                with tc.tile_pool(name=f"psv{b}", bufs=2, space="PSUM") as psv:
                    for i in range(16):
                        vp = psv.tile([128, COLS], F32, name="vp", tag="vp")
                        for k in range(KT):
                            nc.tensor.matmul(
                                vp, lhsT=xt[:, k, i * 128:(i + 1) * 128],
                                rhs=wv_sb[:, k, :], start=(k == 0), stop=(k == KT - 1))
                        nc.scalar.copy(out=vv[:, i, :], in_=vp)

                # ---------- q/k projections, N=512, fused bias + RoPE ----
                with tc.tile_pool(name=f"psqk{b}", bufs=3, space="PSUM") as psqk:
                    for wsb, bsb, dst in ((wq_sb, bq_sb, qT), (wk_sb, bk_sb, kT)):
                        for h in range(HPC):
                            for t in range(TC4):
                                ts = slice(t * 512, (t + 1) * 512)
                                pp = psqk.tile([128, 512], F32, name="pp", tag="pp")
                                for k in range(KT):
                                    nc.tensor.matmul(
                                        pp, lhsT=wsb[:, k, h * 128:(h + 1) * 128],
                                        rhs=xt[:, k, ts], start=(k == 0),
                                        stop=(k == KT - 1))
                                # biased staging copies: sg = (lo;hi),
                                # sgX = (hi;lo) so every DVE op has both
                                # SBUF inputs base-aligned (walrus rule).
                                sg = stg.tile([128, 512], BF16, name="sg", tag="sg")
                                nc.scalar.activation(
                                    out=sg, in_=pp, func=Ident, bias=bsb[:, h:h + 1])
                                sgX = stg.tile([128, 512], BF16, name="sgX", tag="sgX")
                                nc.scalar.activation(
                                    out=sgX[0:64, :], in_=pp[64:128, :], func=Ident,
                                    bias=bsb[64:128, h:h + 1])
                                nc.scalar.activation(
                                    out=sgX[64:128, :], in_=pp[0:64, :], func=Ident,
                                    bias=bsb[0:64, h:h + 1])
                                # rope(x) = t1 + t2:
                                #   t1 = sin2s*sgX = (-sin*hi ; sin*lo)
                                #   t2 = cos2*sg = (cos*lo ; cos*hi)
                                t1 = ropep.tile([128, 512], BF16, name="t1", tag="t1")
                                nc.vector.tensor_mul(t1, sin_sb[:, ts], sgX)
                                t2 = ropep.tile([128, 512], BF16, name="t2", tag="t2")
                                nc.vector.tensor_mul(t2, cos_sb[:, ts], sg)
                                nc.vector.tensor_add(dst[h][:, ts], t1, t2)

                # ---------- attention + out-proj per l-chunk ----------
                with (
                    tc.tile_pool(name=f"pst{b}", bufs=2, space="PSUM") as pst,
                    tc.tile_pool(name=f"psav{b}", bufs=2, space="PSUM") as psav,
                    tc.tile_pool(name=f"psrs{b}", bufs=2, space="PSUM") as psrs,
                    tc.tile_pool(name=f"psy{b}", bufs=2, space="PSUM") as psy,
                ):
                    for lc in range(LC):
                        lcs = slice(lc * 512, (lc + 1) * 512)
                        for h in range(HPC):
                            av = psav.tile([128, 512], F32, name="av", tag="av")
                            rs = psrs.tile([1, 512], F32, name="rs", tag="rs")
                            nblk = 4 * lc + 4
                            for mb in range(nblk):
                                l0 = max(lc * 512, mb * 128)
                                npr = lc * 512 + 512 - l0
                                c0 = l0 - lc * 512
                                st = pst.tile([128, 512], F32, name="st", tag="st")
                                nc.tensor.matmul(
                                    st[:, 0:npr],
                                    lhsT=kT[h][:, mb * 128:(mb + 1) * 128],
                                    rhs=qT[h][:, l0:l0 + npr],
                                    start=True, stop=True)
                                pt = ptp.tile([128, 512], BF16, name="pt", tag="pt")
                                nc.scalar.activation(
                                    out=pt[:, 0:npr], in_=st[:, 0:npr], func=Exp)
                                if mb >= 4 * lc:  # diagonal: zero m>l via 0/1 tri
                                    nc.vector.tensor_mul(
                                        pt[:, 0:128], pt[:, 0:128], tri_sb)
                                nc.tensor.matmul(
                                    av[:, c0:512],
                                    lhsT=vv[:, mb, h * 128:(h + 1) * 128],
                                    rhs=pt[:, 0:npr], start=(mb == 0),
                                    stop=(mb == nblk - 1))
                                nc.tensor.matmul(
                                    rs[0:1, c0:512], lhsT=onesb_sb[:, 0:1],
                                    rhs=pt[:, 0:npr], start=(mb == 0),
                                    stop=(mb == nblk - 1))
                            rec = smallp.tile([1, 512], F32R, name="rec", tag="rec")
                            nc.vector.reciprocal(out=rec, in_=rs[0:1, :])
                            bc = pst.tile([128, 512], F32, name="bc", tag="st")
                            nc.tensor.matmul(bc, lhsT=onesr_sb[0:1, :], rhs=rec,
                                             start=True, stop=True)
                            bcs = smallp.tile([128, 512], F32, name="bcs", tag="bcs")
                            nc.scalar.copy(out=bcs, in_=bc)
                            nc.vector.tensor_mul(oT[h][:, lcs], av, bcs)
                        # out-proj for this l-chunk (both heads ready)
                        for eb in range(KT):
                            yp = psy.tile([128, 512], F32, name="yp", tag="yp")
                            for h in range(HPC):
                                nc.tensor.matmul(
                                    yp, lhsT=wo_sb[:, h, eb * 128:(eb + 1) * 128],
                                    rhs=oT[h][:, lcs],
                                    start=(h == 0), stop=(h == HPC - 1))
                            ys = yst.tile([128, 512], BF16, name="ys", tag="ys")
                            if eb % 2 == 0:
                                nc.scalar.copy(out=ys, in_=yp)
                            else:
                                nc.vector.tensor_copy(ys, yp)
                            nc.sync.dma_start(
                                out=y_d[b, eb * 128:(eb + 1) * 128, lcs],
                                in_=ys)
    nc.compile()
    return nc


_NC_CACHE = None


def kernel(x, Wq, bq, Wk, bk, Wv, bv, Wo, bo):
    global _NC_CACHE
    import ml_dtypes
    from concourse.bass_utils import run_bass_kernel_spmd

    BF = ml_dtypes.bfloat16
    x = np.asarray(x, np.float32)
    Wq = np.asarray(Wq, np.float32)
    Wk = np.asarray(Wk, np.float32)
    Wv = np.asarray(Wv, np.float32)
    Wo = np.asarray(Wo, np.float32)
    bq = np.asarray(bq, np.float32)
    bk = np.asarray(bk, np.float32)
    bv = np.asarray(bv, np.float32)
    bo = np.asarray(bo, np.float32)
    scale = HD ** (-0.5)

    inv = 1.0 / (BASE ** (np.arange(0, HD, 2, dtype=np.float32) / HD))
    fr = np.outer(inv, np.arange(L, dtype=np.float32))  # [64, L]
    cosf = np.cos(fr)
    sinf = np.sin(fr)
    cos2 = np.concatenate([cosf, cosf], axis=0).astype(BF)    # [128, L]
    sin2s = np.concatenate([-sinf, sinf], axis=0).astype(BF)  # [128, L]
    tri = (np.arange(128)[:, None] <= np.arange(128)[None, :]).astype(BF)

    xT = np.ascontiguousarray(np.transpose(x, (0, 2, 1))).astype(BF)  # [B, E, L]

    in_maps = []
    for c in range(NCORES):
        cols = slice(c * COLS, (c + 1) * COLS)
        bq_c = (bq[cols] * scale).reshape(HPC, 128).T.copy()  # [128, HPC] f32
        bk_c = bk[cols].reshape(HPC, 128).T.copy()
        in_maps.append({
            "xT": xT,
            "wq": np.ascontiguousarray(Wq[:, cols] * scale).astype(BF),
            "wk": np.ascontiguousarray(Wk[:, cols]).astype(BF),
            "wv": np.ascontiguousarray(Wv[:, cols]).astype(BF),
            "wo": np.ascontiguousarray(Wo[cols, :]).astype(BF),
            "bq": bq_c, "bk": bk_c,
            "cos2": cos2,
            "sin2s": sin2s,
            "tri": tri,
            "onesb": np.ones((128, 1), BF),
            "onesr": np.ones((1, 128), np.float32),
        })

    if _NC_CACHE is None:
        _NC_CACHE = _build_program()
    res = run_bass_kernel_spmd(_NC_CACHE, in_maps, list(range(NCORES)))
    acc = np.zeros((B, E, L), np.float32)
    for c in range(NCORES):
        acc += res.results[c]["yT"].astype(np.float32)
    bo_eff = bo + bv @ Wo  # v-bias folded: softmax rows sum to 1
    y = np.transpose(acc, (0, 2, 1)) + bo_eff
    return y.astype(np.float32)
